# revision 20
# baseline (speedup 1.0000x reference)
"""Trainium2 Bass kernel for a 4-layer dense transformer (nn_Athena_24739011625811).

Strategy (8 NeuronCores, SPMD, fully sequence-sharded / data-parallel):
  - Core c owns tokens [256c, 256c+256) end-to-end.  Residual kept E-major
    ([e, tok]) in SBUF as f32 with an f16 shadow copy (xc) used as matmul
    input.  RMS-norm scale is COMMUTED through the matmuls: projections run
    on the raw residual and the per-token scale is folded into the rope
    tables (q/k), a per-partition scalar multiply (v), the FFN evac path
    (gate/up), and the final na tiles (vocab).  This removes the norm
    serialization from the layer-boundary critical path.
  - All weights are FULL on every core and streamed from HBM, overlapping
    compute.  QKV weights for layer l+1 are prefetched mid-FFN of layer l
    (persistent tiles, sync queue); q weights stream on the scalar queue.
  - Sliding-window attention (window 1024): per layer TWO AllGathers (one
    per pair of kv heads) so the first 8 q heads only wait for the first
    half of the exchange.  Each core then fetches its 4 neighbor blocks
    per group with dynamic-offset DMAs.  Layer 0's kv window is
    host-precomputed (x0 is host-known): no collective at all.
  - Vocab projection is activation-stationary: the normed activations are
    the PE stationary operand and the vocab weights stream with free dim
    512 (two PSUM banks per group), producing token-major logits that the
    host transposes.  This runs the PE at N=512 instead of N=256.
  - Matmuls f16 (f32 PSUM), residual f32, logits f16.
"""

import math

import numpy as np

import concourse.bass as bass
import concourse.mybir as mybir
import concourse.tile as tile
from concourse import bacc
from concourse.bass_utils import run_bass_kernel_spmd

F16 = mybir.dt.float16
F32 = mybir.dt.float32
I32 = mybir.dt.int32
AF = mybir.ActivationFunctionType
ALU = mybir.AluOpType

V, E, HID, L = 32000, 2048, 8192, 4
H, KV, DK = 16, 4, 128
S, WINDOW = 2048, 1024
EPS = 1e-5
NCORES = 8
SL = S // NCORES          # 256 tokens per core
NET = E // 128            # 16 e-tiles
NHT = HID // 128          # 64 hidden tiles
VP = 32768                # padded vocab
NVG = VP // 1024          # 32 vocab groups of 1024 cols
NB = 5                    # 256-token blocks in the attention window
GW = 2 * 512              # bytes.. columns per kv-group block (k|v for 2 heads)
RG = [list(range(NCORES))]

_BUILT = None


def build_graph(layers=L):
    nc = bacc.Bacc("TRN2", target_bir_lowering=False, debug=False, num_devices=NCORES)

    x0_d = nc.declare_dram_parameter("x0", [E, SL], F32, isOutput=False)
    cos_d = nc.declare_dram_parameter("cosT", [128, SL], F32, isOutput=False)
    sin_d = nc.declare_dram_parameter("sinT", [128, SL], F32, isOutput=False)
    mask_d = nc.declare_dram_parameter("masks", [NB, 2, 128, 2 * SL], F16,
                                       isOutput=False)
    nb_d = nc.declare_dram_parameter("nbidx", [1, 8], I32, isOutput=False)
    kvg0_d = nc.declare_dram_parameter("kvg0", [NB - 1, 128, KV * 512], F16,
                                       isOutput=False)
    kvself0_d = nc.declare_dram_parameter("kvself0", [128, KV * 512], F16,
                                          isOutput=False)
    ones_d = nc.declare_dram_parameter("ones", [128, 1], F16, isOutput=False)
    onesr_d = nc.declare_dram_parameter("onesr", [1, 128], F16, isOutput=False)
    wq_d = nc.declare_dram_parameter("wq", [layers, H, 128, E], F16, isOutput=False)
    wk_d = nc.declare_dram_parameter("wk", [layers, KV, 128, E], F16, isOutput=False)
    wv_d = nc.declare_dram_parameter("wv", [layers, KV, 128, E], F16, isOutput=False)
    wo_d = nc.declare_dram_parameter("wo", [layers, NET, 128, E], F16, isOutput=False)
    wup_d = nc.declare_dram_parameter("wup", [layers, 2 * NHT, 128, E], F16,
                                      isOutput=False)
    wdn_d = nc.declare_dram_parameter("wdn", [layers, NET, 128, HID], F16,
                                      isOutput=False)
    wvg_d = nc.declare_dram_parameter("wvg", [NVG, 128, NET * 1024], F16,
                                      isOutput=False)
    out_d = nc.declare_dram_parameter("out", [2, 128, VP], F16, isOutput=True)

    inv_sqrt_dk = float(1.0 / math.sqrt(DK))

    with tile.TileContext(nc) as tc:
        from contextlib import ExitStack

        with ExitStack() as ctx:
            persist = ctx.enter_context(tc.tile_pool(name="persist", bufs=1))
            dcomm = ctx.enter_context(tc.tile_pool(name="dcomm", bufs=2, space="DRAM"))

            # residual x (E-major, f32) + f16 shadow + constants
            x_sb = [persist.tile([128, SL], F32, name=f"x{et}", tag=f"x{et}")
                    for et in range(NET)]
            for et in range(NET):
                nc.sync.dma_start(out=x_sb[et][:],
                                  in_=x0_d[et * 128:(et + 1) * 128, :])

            def xc_tile(et):
                return persist.tile([128, SL], F16, name="xc", tag=f"xc{et}", bufs=2)

            # initial f16 shadow
            xc = [None] * NET
            for et in range(NET):
                xc[et] = xc_tile(et)
                eng = nc.scalar if et % 2 == 0 else nc.vector
                if et % 2 == 0:
                    eng.copy(xc[et][:], x_sb[et][:])
                else:
                    eng.tensor_copy(xc[et][:], x_sb[et][:])

            cos_sb = persist.tile([128, SL], F32, name="cos", tag="cos")
            sin_sb = persist.tile([128, SL], F32, name="sin", tag="sin")
            nc.sync.dma_start(out=cos_sb[:], in_=cos_d[:, :])
            nc.sync.dma_start(out=sin_sb[:], in_=sin_d[:, :])
            ones_sb = persist.tile([128, 1], F16, name="ones", tag="ones")
            nc.sync.dma_start(out=ones_sb[:], in_=ones_d[:, :])
            onesr_sb = persist.tile([1, 128], F16, name="onesr", tag="onesr")
            nc.sync.dma_start(out=onesr_sb[:], in_=onesr_d[:, :])
            eps_sb = persist.tile([1, 1], F32, name="epsc", tag="epsc")
            nc.gpsimd.memset(eps_sb[:], float(EPS))
            ones32_sb = persist.tile([1, 1], F32, name="ones32", tag="ones32")
            nc.gpsimd.memset(ones32_sb[:], 1.0)
            nb_sb = persist.tile([1, 8], I32, name="nbs", tag="nbs")
            nc.sync.dma_start(out=nb_sb[:], in_=nb_d[:, :])
            nb0v = nc.values_load(nb_sb[0:1, 0:1], min_val=0,
                                  max_val=NCORES - NB + 1,
                                  skip_runtime_bounds_check=True)

            # persistent k/v weight tiles, prefetched one layer ahead
            wkP = [persist.tile([128, E], F16, name="wkP", tag=f"wkP{kvh}")
                   for kvh in range(KV)]
            wvP = [persist.tile([128, E], F16, name="wvP", tag=f"wvP{kvh}")
                   for kvh in range(KV)]

            def fetch_kv_weights(l):
                for kvh in range(KV):
                    nc.sync.dma_start(out=wkP[kvh][:], in_=wk_d[l, kvh])
                    nc.sync.dma_start(out=wvP[kvh][:], in_=wv_d[l, kvh])

            fetch_kv_weights(0)
            # (h2, q)-duplicated masks per (block, key-half); loaded inside the
            # layer-0 block so the layer-0 kv fetch wins the sync queue
            mask_sb = [[persist.tile([128, 2 * SL], F16, name=f"mask{i}{a}",
                                     tag=f"mask{i}{a}") for a in range(2)]
                       for i in range(NB)]

            # ---- rms-norm scale: ssum -> r [1,SL] f16 and rb [128,SL] f32 ----
            def norm_scale(sbP, psP, pfx):
                ssum = psP.tile([1, SL], F32, name="ssum", tag=f"ss{pfx}", bufs=1)
                for et in range(NET):
                    sq = sbP.tile([128, SL], F16, name="sq", tag="sq", bufs=3)
                    nc.scalar.activation(sq[:], xc[et][:], AF.Square, scale=0.0625)
                    nc.tensor.matmul(ssum[:], ones_sb[:], sq[:],
                                     start=(et == 0), stop=(et == NET - 1))
                lnm = sbP.tile([1, SL], F32, name="lnm", tag=f"ln{pfx}", bufs=1)
                nc.scalar.activation(lnm[:], ssum[:], AF.Ln,
                                     scale=float(256.0 / E), bias=eps_sb[:])
                r = sbP.tile([1, SL], F16, name="rr", tag=f"rr{pfx}", bufs=1)
                nc.scalar.activation(r[:], lnm[:], AF.Exp, scale=-0.5)
                rbp = psP.tile([128, SL], F32, name="rbp", tag=f"rb{pfx}", bufs=1)
                nc.tensor.matmul(rbp[:], onesr_sb[:], r[:], start=True, stop=True)
                rb = sbP.tile([128, SL], F32, name="rb", tag=f"rc{pfx}", bufs=1)
                nc.scalar.copy(rb[:], rbp[:])
                return r, rb

            def rope(ps, out_ap, sbR, cosm, sinm):
                t0 = sbR.tile([128, SL], F32, name="rt0", tag="rt0", bufs=2)
                nc.vector.tensor_mul(t0[:], ps[:], cosm[:])
                t1 = sbR.tile([128, SL], F32, name="rt1", tag="rt1", bufs=2)
                nc.vector.tensor_mul(t1[0:64, :], ps[64:128, :], sinm[0:64, :])
                nc.vector.tensor_mul(t1[64:128, :], ps[0:64, :], sinm[64:128, :])
                nc.vector.tensor_add(out_ap, t0[:], t1[:])

            for l in range(layers):
                # ======== attention ========
                with tc.tile_pool(name=f"sbA_{l}", bufs=1) as sbA:
                    psA_cm = tc.tile_pool(name=f"psA_{l}", bufs=1, space="PSUM")
                    psA = psA_cm.__enter__()

                    r, rb = norm_scale(sbA, psA, "a")
                    # fold norm scale into the rope tables
                    cosm = persist.tile([128, SL], F32, name="cosm", tag="cosm",
                                        bufs=2)
                    sinm = persist.tile([128, SL], F32, name="sinm", tag="sinm",
                                        bufs=2)
                    nc.vector.tensor_mul(cosm[:], cos_sb[:], rb[:])
                    nc.vector.tensor_mul(sinm[:], sin_sb[:], rb[:])
                    # per-partition token scale for v (transpose of r)
                    r32 = sbA.tile([1, SL], F32, name="r32", tag="r32")
                    nc.scalar.copy(r32[:], r[:])
                    psT = psA.tile([128, 2], F32, name="psT", tag="psT", bufs=1)
                    for tt in range(2):
                        nc.tensor.transpose(psT[:, tt:tt + 1],
                                            r32[0:1, tt * 128:(tt + 1) * 128],
                                            ones32_sb[0:1, 0:1])
                    rbT2 = sbA.tile([128, 2], F32, name="rbT", tag="rbT")
                    nc.scalar.copy(rbT2[:], psT[:])
                    rbT = [rbT2[:, 0:1], rbT2[:, 1:2]]

                    # ---- k, v per kv-group; publish + AllGather per group ----
                    k_loc = [sbA.tile([128, SL], F16, name="kloc", tag=f"kl{i}")
                             for i in range(KV)]
                    v_loc = [sbA.tile([128, SL], F16, name="vloc", tag=f"vl{i}")
                             for i in range(KV)]
                    kvgall_g = [sbA.tile([128, (NB - 1) * GW], F16, name="kvgall",
                                         tag=f"kvgall{g}") for g in range(2)]
                    kv_out = [None, None]
                    if l == 0:
                        for kvh in range(KV):
                            nc.sync.dma_start(
                                out=k_loc[kvh][:],
                                in_=kvself0_d[:, kvh * 512:kvh * 512 + 256])
                            nc.sync.dma_start(
                                out=v_loc[kvh][:],
                                in_=kvself0_d[:, kvh * 512 + 256:kvh * 512 + 512])
                        for g in range(2):
                            for i in range(NB - 1):
                                nc.sync.dma_start(
                                    out=kvgall_g[g][:, i * GW:(i + 1) * GW],
                                    in_=kvg0_d[i, :, g * GW:(g + 1) * GW])
                        for i in range(NB):
                            for a in range(2):
                                nc.sync.dma_start(out=mask_sb[i][a][:],
                                                  in_=mask_d[i, a, :, :])
                    else:
                        for g in range(2):
                            kv_in = dcomm.tile([128, GW], F16, name="kvin",
                                               tag=f"kv_in{g}", bufs=2)
                            for kvh in (2 * g, 2 * g + 1):
                                co = (kvh % 2) * 512
                                psk = psA.tile([128, SL], F32, name="psk",
                                               tag="pqk", bufs=3)
                                for et in range(NET):
                                    nc.tensor.matmul(
                                        psk[:], wkP[kvh][:, et * 128:(et + 1) * 128],
                                        xc[et][:],
                                        start=(et == 0), stop=(et == NET - 1))
                                rope(psk[:], k_loc[kvh][:], sbA, cosm, sinm)
                                nc.gpsimd.dma_start(
                                    out=kv_in[:, co:co + 256], in_=k_loc[kvh][:])
                                for tt in range(2):
                                    psv = psA.tile([128, 128], F32, name="psv",
                                                   tag="psv", bufs=2)
                                    for et in range(NET):
                                        nc.tensor.matmul(
                                            psv[:],
                                            xc[et][:, tt * 128:(tt + 1) * 128],
                                            wvP[kvh][:, et * 128:(et + 1) * 128],
                                            start=(et == 0), stop=(et == NET - 1))
                                    nc.scalar.mul(
                                        v_loc[kvh][:, tt * 128:(tt + 1) * 128],
                                        psv[:], rbT[tt])
                                nc.gpsimd.dma_start(
                                    out=kv_in[:, co + 256:co + 512],
                                    in_=v_loc[kvh][:])
                            ko = dcomm.tile([NCORES, 128, GW], F16, name="kvout",
                                            tag=f"kv_out{g}", bufs=2,
                                            addr_space="Shared")
                            nc.gpsimd.collective_compute(
                                "AllGather", ALU.bypass, replica_groups=RG,
                                ins=[kv_in[:].opt()], outs=[ko[:].opt()])
                            kv_out[g] = ko

                    # ---- q (overlaps the AllGathers); weights on scalar queue.
                    # Heads sharing a kv head are packed in pairs so scores and
                    # AV can run with a 512-wide moving operand. ----
                    qp = [sbA.tile([128, 2 * SL], F16, name="qp", tag=f"qp{p}")
                          for p in range(H // 2)]
                    for h in range(H):
                        wqc = sbA.tile([128, E], F16, name="wqc", tag="wqc", bufs=3)
                        nc.scalar.dma_start(out=wqc[:], in_=wq_d[l, h])
                        psq = psA.tile([128, SL], F32, name="psq", tag="pqk", bufs=3)
                        for et in range(NET):
                            nc.tensor.matmul(psq[:], wqc[:, et * 128:(et + 1) * 128],
                                             xc[et][:],
                                             start=(et == 0), stop=(et == NET - 1))
                        rope(psq[:],
                             qp[h // 2][:, (h % 2) * SL:(h % 2 + 1) * SL],
                             sbA, cosm, sinm)

                    psA_cm.__exit__(None, None, None)
                    psB_cm = tc.tile_pool(name=f"psB_{l}", bufs=1, space="PSUM")
                    psB = psB_cm.__enter__()

                    # ---- per-group neighbor fetch + 4 head-pairs ----
                    attnT = [sbA.tile([128, 2 * SL], F16, name="attnT", tag=f"at{p}")
                             for p in range(H // 2)]
                    for g in range(2):
                        kvgall = kvgall_g[g]
                        if l > 0:
                            for i in range(NB - 1):
                                for hf in range(2):
                                    nc.gpsimd.dma_start(
                                        out=kvgall[:, i * GW + hf * 512:
                                                   i * GW + (hf + 1) * 512],
                                        in_=kv_out[g][bass.ds(nb0v + i, 1), :,
                                                      hf * 512:(hf + 1) * 512])

                        def kvs(i, kvh, off, size):
                            base = i * GW + (kvh % 2) * 512 + off
                            return kvgall[:, base:base + size]

                        # (block, key-half) pairs: own block first, then the
                        # fetched neighbors in arrival order
                        ia = [(NB - 1, 0), (NB - 1, 1)] + [
                            (i, a) for i in range(NB - 1) for a in range(2)]
                        for p in range(4 * g, 4 * g + 4):
                            kvh = p // 2
                            pts = {}
                            for (i, a) in ia:
                                pss = psB.tile([128, 2 * SL], F32, name="pss",
                                               tag="pss", bufs=2)
                                if i == NB - 1:
                                    klhs = k_loc[kvh][:, a * 128:(a + 1) * 128]
                                else:
                                    klhs = kvs(i, kvh, a * 128, 128)
                                nc.tensor.matmul(pss[:], klhs, qp[p][:],
                                                 start=True, stop=True)
                                pt = sbA.tile([128, 2 * SL], F16, name="pt",
                                              tag="pt", bufs=12)
                                nc.scalar.activation(pt[:], pss[:], AF.Exp,
                                                     scale=inv_sqrt_dk)
                                nc.vector.tensor_mul(pt[:], pt[:],
                                                     mask_sb[i][a][:])
                                pts[(i, a)] = pt
                            psl = psB.tile([1, 2 * SL], F32, name="psl", tag="psl",
                                           bufs=2)
                            for j, (i, a) in enumerate(ia):
                                nc.tensor.matmul(psl[:], ones_sb[:],
                                                 pts[(i, a)][:],
                                                 start=(j == 0),
                                                 stop=(j == 2 * NB - 1))
                            psa = psB.tile([128, 2 * SL], F32, name="psa",
                                           tag="psa", bufs=2)
                            for j, (i, a) in enumerate(ia):
                                if i == NB - 1:
                                    vls = v_loc[kvh][:, a * 128:(a + 1) * 128]
                                else:
                                    vls = kvs(i, kvh, 256 + a * 128, 128)
                                nc.tensor.matmul(psa[:], vls, pts[(i, a)][:],
                                                 start=(j == 0),
                                                 stop=(j == 2 * NB - 1))
                            linv = sbA.tile([1, 2 * SL], F16, name="linv",
                                            tag="linv", bufs=2)
                            with nc.allow_low_precision(reason="f16 softmax denom"):
                                nc.vector.reciprocal(linv[:], psl[:])
                            lbp = psB.tile([128, 2 * SL], F32, name="lbp",
                                           tag="pss", bufs=2)
                            nc.tensor.matmul(lbp[:], onesr_sb[:], linv[:],
                                             start=True, stop=True)
                            lbc = sbA.tile([128, 2 * SL], F32, name="lbc",
                                           tag="lbc", bufs=2)
                            nc.scalar.copy(lbc[:], lbp[:])
                            nc.vector.tensor_mul(attnT[p][:], psa[:], lbc[:])

                    # ---- output projection (weights streamed, sync queue) ----
                    for eo in range(NET):
                        woc = sbA.tile([128, E], F16, name="woc", tag="woc", bufs=4)
                        nc.sync.dma_start(out=woc[:], in_=wo_d[l, eo])
                        psy = psB.tile([128, SL], F32, name="psy", tag="psy", bufs=2)
                        for ht in range(H):
                            nc.tensor.matmul(
                                psy[:],
                                woc[:, ht * 128:(ht + 1) * 128],
                                attnT[ht // 2][:, (ht % 2) * SL:(ht % 2 + 1) * SL],
                                start=(ht == 0), stop=(ht == H - 1))
                        nc.vector.tensor_add(x_sb[eo][:], x_sb[eo][:], psy[:])
                        xc[eo] = xc_tile(eo)
                        if eo % 2 == 0:
                            nc.scalar.copy(xc[eo][:], x_sb[eo][:])
                        else:
                            nc.vector.tensor_copy(xc[eo][:], x_sb[eo][:])
                    psB_cm.__exit__(None, None, None)

                # ======== FFN ========
                with tc.tile_pool(name=f"sbF_{l}", bufs=1) as sbF, \
                     tc.tile_pool(name=f"psF_{l}", bufs=1, space="PSUM") as psF:
                    _, rb2 = norm_scale(sbF, psF, "f")
                    hid = [sbF.tile([128, SL], F16, name="hid", tag=f"h{g}")
                           for g in range(NHT)]
                    for g in range(NHT):
                        wgc = sbF.tile([128, E], F16, name="wgc", tag="wgc", bufs=3)
                        nc.sync.dma_start(out=wgc[:], in_=wup_d[l, g])
                        wuc = sbF.tile([128, E], F16, name="wuc", tag="wuc", bufs=3)
                        nc.sync.dma_start(out=wuc[:], in_=wup_d[l, NHT + g])
                        psg = psF.tile([128, SL], F32, name="psg", tag="pgu", bufs=3)
                        for et in range(NET):
                            nc.tensor.matmul(psg[:], wgc[:, et * 128:(et + 1) * 128],
                                             xc[et][:],
                                             start=(et == 0), stop=(et == NET - 1))
                        psu = psF.tile([128, SL], F32, name="psu", tag="pgu", bufs=3)
                        for et in range(NET):
                            nc.tensor.matmul(psu[:], wuc[:, et * 128:(et + 1) * 128],
                                             xc[et][:],
                                             start=(et == 0), stop=(et == NET - 1))
                        # gate/up are unnormalized: scale by rb2 in the evac path
                        nc.vector.tensor_mul(psg[:], psg[:], rb2[:])
                        sg = sbF.tile([128, SL], F16, name="sg", tag="sg", bufs=2)
                        nc.scalar.activation(sg[:], psg[:], AF.Silu)
                        nc.vector.tensor_mul(psu[:], psu[:], rb2[:])
                        nc.vector.tensor_mul(hid[g][:], psu[:], sg[:])
                        if g == 52:
                            # prefetch next layer's k/v weights while the sync
                            # queue still has slack before the down-proj burst
                            if l + 1 < layers:
                                fetch_kv_weights(l + 1)
                    # down-proj
                    for eo in range(NET):
                        wdc = sbF.tile([128, HID], F16, name="wdc", tag="wdc", bufs=3)
                        nc.sync.dma_start(out=wdc[:], in_=wdn_d[l, eo])
                        psd = psF.tile([128, SL], F32, name="psd", tag="psd", bufs=3)
                        for ht in range(NHT):
                            nc.tensor.matmul(psd[:], wdc[:, ht * 128:(ht + 1) * 128],
                                             hid[ht][:],
                                             start=(ht == 0), stop=(ht == NHT - 1))
                        nc.vector.tensor_add(x_sb[eo][:], x_sb[eo][:], psd[:])
                        xc[eo] = xc_tile(eo)
                        if eo % 2 == 0:
                            nc.scalar.copy(xc[eo][:], x_sb[eo][:])
                        else:
                            nc.vector.tensor_copy(xc[eo][:], x_sb[eo][:])

            # ======== final norm + vocab projection (token-major, N=512) ========
            with tc.tile_pool(name="sbV", bufs=1) as sbV, \
                 tc.tile_pool(name="psV", bufs=1, space="PSUM") as psV:
                _, rb3 = norm_scale(sbV, psV, "v")
                na = [persist.tile([128, SL], F16, name="na", tag=f"na{et}")
                      for et in range(NET)]
                for et in range(NET):
                    nc.vector.tensor_mul(na[et][:], x_sb[et][:], rb3[:])
                for vg in range(NVG):
                    # group 31 holds only the 256-col vocab tail, packed at
                    # stride 256 per e-tile in its first 4096 columns
                    last = vg == NVG - 1
                    cw = 256 if last else 512
                    nch = 1 if last else 2
                    wg = sbV.tile([128, NET * 1024], F16, name="wg", tag="wg", bufs=2)
                    weng = nc.sync if vg % 2 == 0 else nc.scalar
                    if last:
                        weng.dma_start(out=wg[:, 0:NET * 256], in_=wvg_d[vg][:, 0:NET * 256])
                    else:
                        weng.dma_start(out=wg[:], in_=wvg_d[vg])
                    for t in range(2):
                        osb = sbV.tile([128, 1024], F16, name="osb", tag="osb",
                                       bufs=4)
                        for c in range(nch):
                            pp = psV.tile([128, 512], F32, name="pp", tag="pp",
                                          bufs=4)
                            for et in range(NET):
                                nc.tensor.matmul(
                                    pp[:, 0:cw],
                                    na[et][:, t * 128:(t + 1) * 128],
                                    wg[:, et * nch * cw + c * cw:
                                       et * nch * cw + (c + 1) * cw],
                                    start=(et == 0), stop=(et == NET - 1))
                            if c == 0:
                                nc.scalar.copy(osb[:, 0:cw], pp[:, 0:cw])
                            else:
                                nc.vector.tensor_copy(osb[:, 512:1024], pp[:])
                        oeng = nc.scalar if vg % 2 == 0 else nc.sync
                        oeng.dma_start(
                            out=out_d[t, :, vg * 1024:vg * 1024 + nch * cw],
                            in_=osb[:, 0:nch * cw])

    nc.compile()
    return nc


# ------------------------------------------------------------------ host side

def _f16(a):
    return np.ascontiguousarray(a).astype(np.float16)


def _rope_tables():
    half = DK // 2
    offs = np.arange(DK) % half
    scales = np.power(10000.0, -2.0 / DK * offs.astype(np.float64))
    ang = np.arange(S, dtype=np.float64)[:, None] * scales[None, :]
    cosT = np.cos(ang).T.astype(np.float32)
    sinT = np.sin(ang).T.astype(np.float32)
    sinT[:half, :] *= -1.0
    return cosT, sinT


def _make_masks(c):
    """[NB, 2(key-half), 128, 2*SL] where the last axis is (head2, q): the
    per-(block, key-half) mask duplicated for both heads of a q-pair.
    Slots 0-3 = contiguous fetched blocks max(0,c-4)+i; slot 4 = own
    block (causal).  A fetched block that is the own block or future is
    fully masked."""
    nb0 = max(0, c - 4)
    masks = np.zeros((NB, 2, 128, 2 * SL), np.float32)
    for pair in range(NB):
        blk = nb0 + pair if pair < NB - 1 else c
        if pair < NB - 1 and blk >= c:
            continue
        for hf in range(2):
            jj = blk * SL + hf * 128 + np.arange(128)[:, None]
            ii = c * SL + np.arange(SL)[None, :]
            m = ((jj <= ii) & (ii - jj < WINDOW)).astype(np.float32)
            masks[pair, hf, :, 0:SL] = m
            masks[pair, hf, :, SL:2 * SL] = m
    return masks


def _prepare_inmaps(tokens, table, wq, wk, wv, wo, w_up, w_down, w_vocab, layers=L):
    tokens = np.asarray(tokens)
    table = np.asarray(table, dtype=np.float32)
    wq = np.asarray(wq, dtype=np.float32)[:layers]
    wk = np.asarray(wk, dtype=np.float32)[:layers]
    wv = np.asarray(wv, dtype=np.float32)[:layers]
    wo = np.asarray(wo, dtype=np.float32)[:layers]
    w_up = np.asarray(w_up, dtype=np.float32)[:layers]
    w_down = np.asarray(w_down, dtype=np.float32)[:layers]
    w_vocab = np.asarray(w_vocab, dtype=np.float32)

    tbl = table.copy()
    tbl[0] = 0.0
    x_full = tbl[tokens[0]]                       # [S, E] f32
    cosT, sinT = _rope_tables()

    # ---- shared packed weights (identical on every core) ----
    wq_p = _f16(wq.reshape(layers, NET, 128, H, 128)
                .transpose(0, 3, 2, 1, 4).reshape(layers, H, 128, E))
    wk_p = _f16(wk.reshape(layers, NET, 128, KV, 128)
                .transpose(0, 3, 2, 1, 4).reshape(layers, KV, 128, E))
    wv_p = _f16(wv.reshape(layers, NET, 128, KV, 128)
                .transpose(0, 3, 2, 1, 4).reshape(layers, KV, 128, E))
    wo_p = _f16(wo.reshape(layers, H, 128, NET, 128)
                .transpose(0, 3, 2, 1, 4).reshape(layers, NET, 128, E))
    gate_p = (w_up[:, :, :HID].reshape(layers, NET, 128, NHT, 128)
              .transpose(0, 3, 2, 1, 4).reshape(layers, NHT, 128, E))
    up_p = (w_up[:, :, HID:].reshape(layers, NET, 128, NHT, 128)
            .transpose(0, 3, 2, 1, 4).reshape(layers, NHT, 128, E))
    wup_p = _f16(np.concatenate([gate_p, up_p], axis=1))
    wdn_p = _f16(w_down.reshape(layers, NHT, 128, NET, 128)
                 .transpose(0, 3, 2, 1, 4).reshape(layers, NET, 128, HID))
    # vocab: [NVG, 128, NET*1024]; wvg[g, p, et*1024+c] = w_vocab[et*128+p, g*1024+c]
    # for g < 31; group 31 packs the 256-col tail at stride 256 per e-tile
    wv_pad = np.zeros((E, VP), np.float32)
    wv_pad[:, :V] = w_vocab
    wvg_p = np.asarray(wv_pad.reshape(NET, 128, NVG, 1024)
                       .transpose(2, 1, 0, 3).reshape(NVG, 128, NET * 1024),
                       dtype=np.float16)
    wvg_p[NVG - 1] = 0.0
    w31 = w_vocab[:, (NVG - 1) * 1024:V]            # [E, 256]
    wvg_p[NVG - 1, :, :NET * 256] = _f16(
        w31.reshape(NET, 128, 256).transpose(1, 0, 2).reshape(128, NET * 256))
    wvg_p = np.ascontiguousarray(wvg_p)
    ones = _f16(np.ones((128, 1), np.float32))
    onesr = _f16(np.ones((1, 128), np.float32))

    # ---- layer-0 kv blocks (host-precomputed, device-matching numerics) ----
    r0 = 1.0 / np.sqrt((x_full.astype(np.float64) ** 2).mean(axis=1) + EPS)
    n0 = (x_full * r0[:, None]).astype(np.float16).astype(np.float32)
    wk0 = wk[0].astype(np.float16).astype(np.float32)
    wv0 = wv[0].astype(np.float16).astype(np.float32)
    half = DK // 2
    kvblk = np.zeros((NCORES, 128, KV * 512), np.float16)
    for b in range(NCORES):
        nb_ = n0[b * SL:(b + 1) * SL]                       # [SL, E]
        cosb = cosT[:, b * SL:(b + 1) * SL]
        sinb = sinT[:, b * SL:(b + 1) * SL]
        for kvh in range(KV):
            kf = (nb_ @ wk0[:, kvh * DK:(kvh + 1) * DK]).T  # [DK, SL]
            t1 = np.empty_like(kf)
            t1[:half] = kf[half:] * sinb[:half]
            t1[half:] = kf[:half] * sinb[half:]
            kr = cosb * kf + t1
            vf = nb_ @ wv0[:, kvh * DK:(kvh + 1) * DK]      # [SL, DK]
            vp_ = vf.reshape(2, 128, 128).transpose(1, 0, 2).reshape(128, 256)
            kvblk[b, :, kvh * 512:kvh * 512 + 256] = kr.astype(np.float16)
            kvblk[b, :, kvh * 512 + 256:kvh * 512 + 512] = vp_.astype(np.float16)

    in_maps = []
    for c in range(NCORES):
        nb0 = max(0, c - 4)
        nbarr = np.zeros((1, 8), np.int32)
        nbarr[0, 0] = nb0
        in_maps.append({
            "x0": np.ascontiguousarray(x_full[SL * c:SL * (c + 1)].T),
            "cosT": np.ascontiguousarray(cosT[:, SL * c:SL * (c + 1)]),
            "sinT": np.ascontiguousarray(sinT[:, SL * c:SL * (c + 1)]),
            "masks": _f16(_make_masks(c)),
            "nbidx": nbarr,
            "kvg0": np.stack([kvblk[nb0 + i] for i in range(NB - 1)]),
            "kvself0": kvblk[c],
            "ones": ones,
            "onesr": onesr,
            "wq": wq_p,
            "wk": wk_p,
            "wv": wv_p,
            "wo": wo_p,
            "wup": wup_p,
            "wdn": wdn_p,
            "wvg": wvg_p,
        })
    return in_maps


def _run(inputs, trace=False, layers=L):
    global _BUILT
    if _BUILT is None or _BUILT[1] != layers:
        _BUILT = (build_graph(layers), layers)
    nc = _BUILT[0]
    in_maps = _prepare_inmaps(layers=layers, **inputs)
    res = run_bass_kernel_spmd(nc, in_maps, core_ids=list(range(NCORES)), trace=trace)
    logits = np.concatenate(
        [res.results[c]["out"].reshape(2 * 128, VP)[:, :V] for c in range(NCORES)],
        axis=0)
    return logits[None].astype(np.float32), res


def kernel(**inputs):
    logits, _ = _run(inputs, trace=False)
    return logits


# revision 29
# speedup vs baseline: 1.0128x; 1.0128x over previous
"""Trainium2 Bass kernel for a 4-layer dense transformer (nn_Athena_24739011625811).

Strategy (8 NeuronCores, SPMD, fully sequence-sharded / data-parallel):
  - Core c owns tokens [256c, 256c+256) end-to-end.  Residual kept E-major
    ([e, tok]) in SBUF as f32 with an f16 shadow copy (xc) used as matmul
    input.  RMS-norm scale is COMMUTED through the matmuls: projections run
    on the raw residual and the per-token scale is folded into the rope
    tables (q/k), a per-partition scalar multiply (v), the FFN evac path
    (gate/up), and the final na tiles (vocab).  This removes the norm
    serialization from the layer-boundary critical path.
  - All weights are FULL on every core and streamed from HBM, overlapping
    compute.  QKV weights for layer l+1 are prefetched mid-FFN of layer l
    (persistent tiles, sync queue); q weights stream on the scalar queue.
  - Sliding-window attention (window 1024): per layer TWO AllGathers (one
    per pair of kv heads) so the first 8 q heads only wait for the first
    half of the exchange.  Each core then fetches its 4 neighbor blocks
    per group with dynamic-offset DMAs.  Layer 0's kv window is
    host-precomputed (x0 is host-known): no collective at all.
  - Vocab projection is activation-stationary: the normed activations are
    the PE stationary operand and the vocab weights stream with free dim
    512 (two PSUM banks per group), producing token-major logits that the
    host transposes.  This runs the PE at N=512 instead of N=256.
  - Matmuls f16 (f32 PSUM), residual f32, logits f16.
"""

import math

import numpy as np

import concourse.bass as bass
import concourse.mybir as mybir
import concourse.tile as tile
from concourse import bacc
from concourse.bass_utils import run_bass_kernel_spmd

F16 = mybir.dt.float16
F32 = mybir.dt.float32
I32 = mybir.dt.int32
AF = mybir.ActivationFunctionType
ALU = mybir.AluOpType

V, E, HID, L = 32000, 2048, 8192, 4
H, KV, DK = 16, 4, 128
S, WINDOW = 2048, 1024
EPS = 1e-5
NCORES = 8
SL = S // NCORES          # 256 tokens per core
NET = E // 128            # 16 e-tiles
NHT = HID // 128          # 64 hidden tiles
VP = 32768                # padded vocab
NVG = VP // 1024          # 32 vocab groups of 1024 cols
NB = 5                    # 256-token blocks in the attention window
GW = 2 * 512              # bytes.. columns per kv-group block (k|v for 2 heads)
RG = [list(range(NCORES))]

_BUILT = None


def build_graph(layers=L):
    nc = bacc.Bacc("TRN2", target_bir_lowering=False, debug=False, num_devices=NCORES)

    x0_d = nc.declare_dram_parameter("x0", [E, SL], F32, isOutput=False)
    cos_d = nc.declare_dram_parameter("cosT", [128, SL], F32, isOutput=False)
    sin_d = nc.declare_dram_parameter("sinT", [128, SL], F32, isOutput=False)
    mask_d = nc.declare_dram_parameter("masks", [NB, 2, 128, 2 * SL], F16,
                                       isOutput=False)
    nb_d = nc.declare_dram_parameter("nbidx", [1, 8], I32, isOutput=False)
    kvg0_d = nc.declare_dram_parameter("kvg0", [NB - 1, 128, KV * 512], F16,
                                       isOutput=False)
    kvself0_d = nc.declare_dram_parameter("kvself0", [128, KV * 512], F16,
                                          isOutput=False)
    ones_d = nc.declare_dram_parameter("ones", [128, 1], F16, isOutput=False)
    onesr_d = nc.declare_dram_parameter("onesr", [1, 128], F16, isOutput=False)
    wq_d = nc.declare_dram_parameter("wq", [layers, H, 128, E], F16, isOutput=False)
    wk_d = nc.declare_dram_parameter("wk", [layers, KV, 128, E], F16, isOutput=False)
    wv_d = nc.declare_dram_parameter("wv", [layers, KV, 128, E], F16, isOutput=False)
    wo_d = nc.declare_dram_parameter("wo", [layers, NET, 128, E], F16, isOutput=False)
    wup_d = nc.declare_dram_parameter("wup", [layers, 2 * NHT, 128, E], F16,
                                      isOutput=False)
    wdn_d = nc.declare_dram_parameter("wdn", [layers, NET, 128, HID], F16,
                                      isOutput=False)
    wvg_d = nc.declare_dram_parameter("wvg", [NVG, 128, NET * 1024], F16,
                                      isOutput=False)
    out_d = nc.declare_dram_parameter("out", [2, 128, VP], F16, isOutput=True)

    inv_sqrt_dk = float(1.0 / math.sqrt(DK))

    with tile.TileContext(nc) as tc:
        from contextlib import ExitStack

        with ExitStack() as ctx:
            persist = ctx.enter_context(tc.tile_pool(name="persist", bufs=1))
            dcomm = ctx.enter_context(tc.tile_pool(name="dcomm", bufs=2, space="DRAM"))

            # residual x (E-major, f32) + f16 shadow + constants
            x_sb = [persist.tile([128, SL], F32, name=f"x{et}", tag=f"x{et}")
                    for et in range(NET)]
            for et in range(NET):
                nc.sync.dma_start(out=x_sb[et][:],
                                  in_=x0_d[et * 128:(et + 1) * 128, :])

            def xc_tile(et):
                return persist.tile([128, SL], F16, name="xc", tag=f"xc{et}", bufs=2)

            # initial f16 shadow
            xc = [None] * NET
            for et in range(NET):
                xc[et] = xc_tile(et)
                eng = nc.scalar if et % 2 == 0 else nc.vector
                if et % 2 == 0:
                    eng.copy(xc[et][:], x_sb[et][:])
                else:
                    eng.tensor_copy(xc[et][:], x_sb[et][:])

            cos_sb = persist.tile([128, SL], F32, name="cos", tag="cos")
            sin_sb = persist.tile([128, SL], F32, name="sin", tag="sin")
            nc.sync.dma_start(out=cos_sb[:], in_=cos_d[:, :])
            nc.sync.dma_start(out=sin_sb[:], in_=sin_d[:, :])
            ones_sb = persist.tile([128, 1], F16, name="ones", tag="ones")
            nc.sync.dma_start(out=ones_sb[:], in_=ones_d[:, :])
            onesr_sb = persist.tile([1, 128], F16, name="onesr", tag="onesr")
            nc.sync.dma_start(out=onesr_sb[:], in_=onesr_d[:, :])
            eps_sb = persist.tile([1, 1], F32, name="epsc", tag="epsc")
            nc.gpsimd.memset(eps_sb[:], float(EPS))
            ones32_sb = persist.tile([1, 1], F32, name="ones32", tag="ones32")
            nc.gpsimd.memset(ones32_sb[:], 1.0)
            nb_sb = persist.tile([1, 8], I32, name="nbs", tag="nbs")
            nc.sync.dma_start(out=nb_sb[:], in_=nb_d[:, :])
            nb0v = nc.values_load(nb_sb[0:1, 0:1], min_val=0,
                                  max_val=NCORES - NB + 1,
                                  skip_runtime_bounds_check=True)

            # persistent k/v weight tiles, prefetched one layer ahead
            wkP = [persist.tile([128, E], F16, name="wkP", tag=f"wkP{kvh}")
                   for kvh in range(KV)]
            wvP = [persist.tile([128, E], F16, name="wvP", tag=f"wvP{kvh}")
                   for kvh in range(KV)]

            def fetch_kv_weights(l):
                for kvh in range(KV):
                    nc.sync.dma_start(out=wkP[kvh][:], in_=wk_d[l, kvh])
                    nc.sync.dma_start(out=wvP[kvh][:], in_=wv_d[l, kvh])

            fetch_kv_weights(0)
            # (h2, q)-duplicated masks per (block, key-half); loaded inside the
            # layer-0 block so the layer-0 kv fetch wins the sync queue
            mask_sb = [[persist.tile([128, 2 * SL], F16, name=f"mask{i}{a}",
                                     tag=f"mask{i}{a}") for a in range(2)]
                       for i in range(NB)]

            # ---- rms-norm scale, split so the broadcast matmul can be
            # emitted AFTER the first consumer chain (tensor queue is FIFO:
            # an early rbp would stall everything behind it) ----
            def norm_start(sbP, psP, pfx):
                ssum = psP.tile([1, SL], F32, name="ssum", tag=f"ss{pfx}", bufs=1)
                for et in range(NET):
                    sq = sbP.tile([128, SL], F16, name="sq", tag="sq", bufs=3)
                    nc.scalar.activation(sq[:], xc[et][:], AF.Square, scale=0.0625)
                    nc.tensor.matmul(ssum[:], ones_sb[:], sq[:],
                                     start=(et == 0), stop=(et == NET - 1))
                lnm = sbP.tile([1, SL], F32, name="lnm", tag=f"ln{pfx}", bufs=1)
                nc.scalar.activation(lnm[:], ssum[:], AF.Ln,
                                     scale=float(256.0 / E), bias=eps_sb[:])
                r = sbP.tile([1, SL], F16, name="rr", tag=f"rr{pfx}", bufs=1)
                nc.scalar.activation(r[:], lnm[:], AF.Exp, scale=-0.5)
                return r

            def norm_finish(r, sbP, psP, pfx):
                rbp = psP.tile([128, SL], F32, name="rbp", tag=f"rb{pfx}", bufs=1)
                nc.tensor.matmul(rbp[:], onesr_sb[:], r[:], start=True, stop=True)
                rb = sbP.tile([128, SL], F32, name="rb", tag=f"rc{pfx}", bufs=1)
                nc.scalar.copy(rb[:], rbp[:])
                return rb

            def rope(ps, out_ap, sbR, cosm, sinm):
                t0 = sbR.tile([128, SL], F32, name="rt0", tag="rt0", bufs=2)
                nc.vector.tensor_mul(t0[:], ps[:], cosm[:])
                t1 = sbR.tile([128, SL], F32, name="rt1", tag="rt1", bufs=2)
                nc.vector.tensor_mul(t1[0:64, :], ps[64:128, :], sinm[0:64, :])
                nc.vector.tensor_mul(t1[64:128, :], ps[0:64, :], sinm[64:128, :])
                nc.vector.tensor_add(out_ap, t0[:], t1[:])

            for l in range(layers):
                # ======== attention ========
                with tc.tile_pool(name=f"sbA_{l}", bufs=1) as sbA:
                    psA_cm = tc.tile_pool(name=f"psA_{l}", bufs=1, space="PSUM")
                    psA = psA_cm.__enter__()

                    r = norm_start(sbA, psA, "a")
                    cosm = persist.tile([128, SL], F32, name="cosm", tag="cosm",
                                        bufs=2)
                    sinm = persist.tile([128, SL], F32, name="sinm", tag="sinm",
                                        bufs=2)

                    def finish_tables():
                        # norm broadcast + rope-table fold + per-partition v
                        # scale; called after the first projection chain so the
                        # rbp matmul doesn't head-block the tensor queue
                        rb = norm_finish(r, sbA, psA, "a")
                        nc.vector.tensor_mul(cosm[:], cos_sb[:], rb[:])
                        nc.vector.tensor_mul(sinm[:], sin_sb[:], rb[:])
                        r32 = sbA.tile([1, SL], F32, name="r32", tag="r32")
                        nc.scalar.copy(r32[:], r[:])
                        psT = psA.tile([128, 2], F32, name="psT", tag="psT",
                                       bufs=1)
                        for tt in range(2):
                            nc.tensor.transpose(psT[:, tt:tt + 1],
                                                r32[0:1, tt * 128:(tt + 1) * 128],
                                                ones32_sb[0:1, 0:1])
                        rbT2 = sbA.tile([128, 2], F32, name="rbT", tag="rbT")
                        nc.scalar.copy(rbT2[:], psT[:])
                        return [rbT2[:, 0:1], rbT2[:, 1:2]]

                    # ---- k, v per kv-group; publish + AllGather per group ----
                    k_loc = [sbA.tile([128, SL], F16, name="kloc", tag=f"kl{i}")
                             for i in range(KV)]
                    v_loc = [sbA.tile([128, SL], F16, name="vloc", tag=f"vl{i}")
                             for i in range(KV)]
                    kvgall_g = [sbA.tile([128, (NB - 1) * GW], F16, name="kvgall",
                                         tag=f"kvgall{g}") for g in range(2)]
                    kv_out = [None, None]
                    rbT = None
                    if l == 0:
                        for kvh in range(KV):
                            nc.sync.dma_start(
                                out=k_loc[kvh][:],
                                in_=kvself0_d[:, kvh * 512:kvh * 512 + 256])
                            nc.sync.dma_start(
                                out=v_loc[kvh][:],
                                in_=kvself0_d[:, kvh * 512 + 256:kvh * 512 + 512])
                        for g in range(2):
                            for i in range(NB - 1):
                                nc.sync.dma_start(
                                    out=kvgall_g[g][:, i * GW:(i + 1) * GW],
                                    in_=kvg0_d[i, :, g * GW:(g + 1) * GW])
                        for i in range(NB):
                            for a in range(2):
                                nc.sync.dma_start(out=mask_sb[i][a][:],
                                                  in_=mask_d[i, a, :, :])
                    else:
                        for g in range(2):
                            kv_in = dcomm.tile([128, GW], F16, name="kvin",
                                               tag=f"kv_in{g}", bufs=2)
                            for kvh in (2 * g, 2 * g + 1):
                                co = (kvh % 2) * 512
                                psk = psA.tile([128, SL], F32, name="psk",
                                               tag="pqk", bufs=3)
                                for et in range(NET):
                                    nc.tensor.matmul(
                                        psk[:], wkP[kvh][:, et * 128:(et + 1) * 128],
                                        xc[et][:],
                                        start=(et == 0), stop=(et == NET - 1))
                                if rbT is None:
                                    rbT = finish_tables()
                                rope(psk[:], k_loc[kvh][:], sbA, cosm, sinm)
                                nc.gpsimd.dma_start(
                                    out=kv_in[:, co:co + 256], in_=k_loc[kvh][:])
                                for tt in range(2):
                                    psv = psA.tile([128, 128], F32, name="psv",
                                                   tag="psv", bufs=2)
                                    for et in range(NET):
                                        nc.tensor.matmul(
                                            psv[:],
                                            xc[et][:, tt * 128:(tt + 1) * 128],
                                            wvP[kvh][:, et * 128:(et + 1) * 128],
                                            start=(et == 0), stop=(et == NET - 1))
                                    nc.scalar.mul(
                                        v_loc[kvh][:, tt * 128:(tt + 1) * 128],
                                        psv[:], rbT[tt])
                                nc.gpsimd.dma_start(
                                    out=kv_in[:, co + 256:co + 512],
                                    in_=v_loc[kvh][:])
                            ko = dcomm.tile([NCORES, 128, GW], F16, name="kvout",
                                            tag=f"kv_out{g}", bufs=2,
                                            addr_space="Shared")
                            nc.gpsimd.collective_compute(
                                "AllGather", ALU.bypass, replica_groups=RG,
                                ins=[kv_in[:].opt()], outs=[ko[:].opt()])
                            kv_out[g] = ko

                    # ---- q (overlaps the AllGathers); weights on scalar queue.
                    # Heads sharing a kv head are packed in pairs so scores and
                    # AV can run with a 512-wide moving operand. ----
                    qp = [sbA.tile([128, 2 * SL], F16, name="qp", tag=f"qp{p}")
                          for p in range(H // 2)]
                    for h in range(H):
                        wqc = sbA.tile([128, E], F16, name="wqc", tag="wqc", bufs=3)
                        nc.scalar.dma_start(out=wqc[:], in_=wq_d[l, h])
                        psq = psA.tile([128, SL], F32, name="psq", tag="pqk", bufs=3)
                        for et in range(NET):
                            nc.tensor.matmul(psq[:], wqc[:, et * 128:(et + 1) * 128],
                                             xc[et][:],
                                             start=(et == 0), stop=(et == NET - 1))
                        if rbT is None:
                            rbT = finish_tables()
                        rope(psq[:],
                             qp[h // 2][:, (h % 2) * SL:(h % 2 + 1) * SL],
                             sbA, cosm, sinm)

                    psA_cm.__exit__(None, None, None)
                    psB_cm = tc.tile_pool(name=f"psB_{l}", bufs=1, space="PSUM")
                    psB = psB_cm.__enter__()

                    # ---- per-group neighbor fetch + 4 head-pairs ----
                    attnT = [sbA.tile([128, 2 * SL], F16, name="attnT", tag=f"at{p}")
                             for p in range(H // 2)]

                    def normalize_pair(p, psl, psa):
                        # softmax denominator + scale; pipelined one pair
                        # behind so the lbp matmul never head-blocks the
                        # tensor queue on the vector reciprocal
                        linv = sbA.tile([1, 2 * SL], F16, name="linv",
                                        tag="linv", bufs=2)
                        with nc.allow_low_precision(reason="f16 softmax denom"):
                            nc.vector.reciprocal(linv[:], psl[:])
                        lbp = psB.tile([128, 2 * SL], F32, name="lbp",
                                       tag="pss", bufs=2)
                        nc.tensor.matmul(lbp[:], onesr_sb[:], linv[:],
                                         start=True, stop=True)
                        lbc = sbA.tile([128, 2 * SL], F32, name="lbc",
                                       tag="lbc", bufs=2)
                        nc.scalar.copy(lbc[:], lbp[:])
                        nc.vector.tensor_mul(attnT[p][:], psa[:], lbc[:])

                    pend = None
                    for g in range(2):
                        kvgall = kvgall_g[g]
                        if l > 0:
                            for i in range(NB - 1):
                                for hf in range(2):
                                    nc.gpsimd.dma_start(
                                        out=kvgall[:, i * GW + hf * 512:
                                                   i * GW + (hf + 1) * 512],
                                        in_=kv_out[g][bass.ds(nb0v + i, 1), :,
                                                      hf * 512:(hf + 1) * 512])

                        def kvs(i, kvh, off, size):
                            base = i * GW + (kvh % 2) * 512 + off
                            return kvgall[:, base:base + size]

                        # (block, key-half) pairs: own block first, then the
                        # fetched neighbors in arrival order
                        ia = [(NB - 1, 0), (NB - 1, 1)] + [
                            (i, a) for i in range(NB - 1) for a in range(2)]
                        for p in range(4 * g, 4 * g + 4):
                            kvh = p // 2
                            pts = {}
                            for (i, a) in ia:
                                pss = psB.tile([128, 2 * SL], F32, name="pss",
                                               tag="pss", bufs=2)
                                if i == NB - 1:
                                    klhs = k_loc[kvh][:, a * 128:(a + 1) * 128]
                                else:
                                    klhs = kvs(i, kvh, a * 128, 128)
                                nc.tensor.matmul(pss[:], klhs, qp[p][:],
                                                 start=True, stop=True)
                                pt = sbA.tile([128, 2 * SL], F16, name="pt",
                                              tag="pt", bufs=12)
                                nc.scalar.activation(pt[:], pss[:], AF.Exp,
                                                     scale=inv_sqrt_dk)
                                nc.vector.tensor_mul(pt[:], pt[:],
                                                     mask_sb[i][a][:])
                                pts[(i, a)] = pt
                            psl = psB.tile([1, 2 * SL], F32, name="psl", tag="psl",
                                           bufs=2)
                            for j, (i, a) in enumerate(ia):
                                nc.tensor.matmul(psl[:], ones_sb[:],
                                                 pts[(i, a)][:],
                                                 start=(j == 0),
                                                 stop=(j == 2 * NB - 1))
                            psa = psB.tile([128, 2 * SL], F32, name="psa",
                                           tag="psa", bufs=2)
                            for j, (i, a) in enumerate(ia):
                                if i == NB - 1:
                                    vls = v_loc[kvh][:, a * 128:(a + 1) * 128]
                                else:
                                    vls = kvs(i, kvh, 256 + a * 128, 128)
                                nc.tensor.matmul(psa[:], vls, pts[(i, a)][:],
                                                 start=(j == 0),
                                                 stop=(j == 2 * NB - 1))
                            if pend is not None:
                                normalize_pair(*pend)
                            pend = (p, psl, psa)
                    normalize_pair(*pend)

                    # ---- output projection (weights streamed, sync queue) ----
                    for eo in range(NET):
                        woc = sbA.tile([128, E], F16, name="woc", tag="woc", bufs=4)
                        nc.sync.dma_start(out=woc[:], in_=wo_d[l, eo])
                        psy = psB.tile([128, SL], F32, name="psy", tag="psy", bufs=2)
                        for ht in range(H):
                            nc.tensor.matmul(
                                psy[:],
                                woc[:, ht * 128:(ht + 1) * 128],
                                attnT[ht // 2][:, (ht % 2) * SL:(ht % 2 + 1) * SL],
                                start=(ht == 0), stop=(ht == H - 1))
                        nc.vector.tensor_add(x_sb[eo][:], x_sb[eo][:], psy[:])
                        xc[eo] = xc_tile(eo)
                        if eo % 2 == 0:
                            nc.scalar.copy(xc[eo][:], x_sb[eo][:])
                        else:
                            nc.vector.tensor_copy(xc[eo][:], x_sb[eo][:])
                    psB_cm.__exit__(None, None, None)

                # ======== FFN ========
                with tc.tile_pool(name=f"sbF_{l}", bufs=1) as sbF, \
                     tc.tile_pool(name=f"psF_{l}", bufs=1, space="PSUM") as psF:
                    r2 = norm_start(sbF, psF, "f")
                    rb2 = None
                    hid = [sbF.tile([128, SL], F16, name="hid", tag=f"h{g}")
                           for g in range(NHT)]
                    for g in range(NHT):
                        wgc = sbF.tile([128, E], F16, name="wgc", tag="wgc", bufs=3)
                        nc.sync.dma_start(out=wgc[:], in_=wup_d[l, g])
                        wuc = sbF.tile([128, E], F16, name="wuc", tag="wuc", bufs=3)
                        nc.sync.dma_start(out=wuc[:], in_=wup_d[l, NHT + g])
                        psg = psF.tile([128, SL], F32, name="psg", tag="pgu", bufs=3)
                        for et in range(NET):
                            nc.tensor.matmul(psg[:], wgc[:, et * 128:(et + 1) * 128],
                                             xc[et][:],
                                             start=(et == 0), stop=(et == NET - 1))
                        psu = psF.tile([128, SL], F32, name="psu", tag="pgu", bufs=3)
                        for et in range(NET):
                            nc.tensor.matmul(psu[:], wuc[:, et * 128:(et + 1) * 128],
                                             xc[et][:],
                                             start=(et == 0), stop=(et == NET - 1))
                        if rb2 is None:
                            rb2 = norm_finish(r2, sbF, psF, "f")
                        # gate/up are unnormalized: scale by rb2 in the evac path
                        nc.vector.tensor_mul(psg[:], psg[:], rb2[:])
                        sg = sbF.tile([128, SL], F16, name="sg", tag="sg", bufs=2)
                        nc.scalar.activation(sg[:], psg[:], AF.Silu)
                        nc.vector.tensor_mul(psu[:], psu[:], rb2[:])
                        nc.vector.tensor_mul(hid[g][:], psu[:], sg[:])
                        if g == 52:
                            # prefetch next layer's k/v weights while the sync
                            # queue still has slack before the down-proj burst
                            if l + 1 < layers:
                                fetch_kv_weights(l + 1)
                    # down-proj
                    for eo in range(NET):
                        wdc = sbF.tile([128, HID], F16, name="wdc", tag="wdc", bufs=3)
                        nc.sync.dma_start(out=wdc[:], in_=wdn_d[l, eo])
                        psd = psF.tile([128, SL], F32, name="psd", tag="psd", bufs=3)
                        for ht in range(NHT):
                            nc.tensor.matmul(psd[:], wdc[:, ht * 128:(ht + 1) * 128],
                                             hid[ht][:],
                                             start=(ht == 0), stop=(ht == NHT - 1))
                        nc.vector.tensor_add(x_sb[eo][:], x_sb[eo][:], psd[:])
                        xc[eo] = xc_tile(eo)
                        if eo % 2 == 0:
                            nc.scalar.copy(xc[eo][:], x_sb[eo][:])
                        else:
                            nc.vector.tensor_copy(xc[eo][:], x_sb[eo][:])

            # ======== final norm + vocab projection (token-major, N=512) ========
            with tc.tile_pool(name="sbV", bufs=1) as sbV, \
                 tc.tile_pool(name="psV", bufs=1, space="PSUM") as psV:
                r3 = norm_start(sbV, psV, "v")
                rb3 = norm_finish(r3, sbV, psV, "v")
                na = [persist.tile([128, SL], F16, name="na", tag=f"na{et}")
                      for et in range(NET)]
                for et in range(NET):
                    nc.vector.tensor_mul(na[et][:], x_sb[et][:], rb3[:])
                for vg in range(NVG):
                    # group 31 holds only the 256-col vocab tail, packed at
                    # stride 256 per e-tile in its first 4096 columns
                    last = vg == NVG - 1
                    cw = 256 if last else 512
                    nch = 1 if last else 2
                    wg = sbV.tile([128, NET * 1024], F16, name="wg", tag="wg", bufs=2)
                    weng = nc.sync if vg % 2 == 0 else nc.scalar
                    if last:
                        weng.dma_start(out=wg[:, 0:NET * 256], in_=wvg_d[vg][:, 0:NET * 256])
                    else:
                        weng.dma_start(out=wg[:], in_=wvg_d[vg])
                    for t in range(2):
                        osb = sbV.tile([128, 1024], F16, name="osb", tag="osb",
                                       bufs=4)
                        for c in range(nch):
                            pp = psV.tile([128, 512], F32, name="pp", tag="pp",
                                          bufs=4)
                            for et in range(NET):
                                nc.tensor.matmul(
                                    pp[:, 0:cw],
                                    na[et][:, t * 128:(t + 1) * 128],
                                    wg[:, et * nch * cw + c * cw:
                                       et * nch * cw + (c + 1) * cw],
                                    start=(et == 0), stop=(et == NET - 1))
                            if c == 0:
                                nc.scalar.copy(osb[:, 0:cw], pp[:, 0:cw])
                            else:
                                nc.vector.tensor_copy(osb[:, 512:1024], pp[:])
                        oeng = nc.scalar if vg % 2 == 0 else nc.sync
                        oeng.dma_start(
                            out=out_d[t, :, vg * 1024:vg * 1024 + nch * cw],
                            in_=osb[:, 0:nch * cw])

    nc.compile()
    return nc


# ------------------------------------------------------------------ host side

def _f16(a):
    return np.ascontiguousarray(a).astype(np.float16)


def _rope_tables():
    half = DK // 2
    offs = np.arange(DK) % half
    scales = np.power(10000.0, -2.0 / DK * offs.astype(np.float64))
    ang = np.arange(S, dtype=np.float64)[:, None] * scales[None, :]
    cosT = np.cos(ang).T.astype(np.float32)
    sinT = np.sin(ang).T.astype(np.float32)
    sinT[:half, :] *= -1.0
    return cosT, sinT


def _make_masks(c):
    """[NB, 2(key-half), 128, 2*SL] where the last axis is (head2, q): the
    per-(block, key-half) mask duplicated for both heads of a q-pair.
    Slots 0-3 = contiguous fetched blocks max(0,c-4)+i; slot 4 = own
    block (causal).  A fetched block that is the own block or future is
    fully masked."""
    nb0 = max(0, c - 4)
    masks = np.zeros((NB, 2, 128, 2 * SL), np.float32)
    for pair in range(NB):
        blk = nb0 + pair if pair < NB - 1 else c
        if pair < NB - 1 and blk >= c:
            continue
        for hf in range(2):
            jj = blk * SL + hf * 128 + np.arange(128)[:, None]
            ii = c * SL + np.arange(SL)[None, :]
            m = ((jj <= ii) & (ii - jj < WINDOW)).astype(np.float32)
            masks[pair, hf, :, 0:SL] = m
            masks[pair, hf, :, SL:2 * SL] = m
    return masks


def _prepare_inmaps(tokens, table, wq, wk, wv, wo, w_up, w_down, w_vocab, layers=L):
    tokens = np.asarray(tokens)
    table = np.asarray(table, dtype=np.float32)
    wq = np.asarray(wq, dtype=np.float32)[:layers]
    wk = np.asarray(wk, dtype=np.float32)[:layers]
    wv = np.asarray(wv, dtype=np.float32)[:layers]
    wo = np.asarray(wo, dtype=np.float32)[:layers]
    w_up = np.asarray(w_up, dtype=np.float32)[:layers]
    w_down = np.asarray(w_down, dtype=np.float32)[:layers]
    w_vocab = np.asarray(w_vocab, dtype=np.float32)

    tbl = table.copy()
    tbl[0] = 0.0
    x_full = tbl[tokens[0]]                       # [S, E] f32
    cosT, sinT = _rope_tables()

    # ---- shared packed weights (identical on every core) ----
    wq_p = _f16(wq.reshape(layers, NET, 128, H, 128)
                .transpose(0, 3, 2, 1, 4).reshape(layers, H, 128, E))
    wk_p = _f16(wk.reshape(layers, NET, 128, KV, 128)
                .transpose(0, 3, 2, 1, 4).reshape(layers, KV, 128, E))
    wv_p = _f16(wv.reshape(layers, NET, 128, KV, 128)
                .transpose(0, 3, 2, 1, 4).reshape(layers, KV, 128, E))
    wo_p = _f16(wo.reshape(layers, H, 128, NET, 128)
                .transpose(0, 3, 2, 1, 4).reshape(layers, NET, 128, E))
    gate_p = (w_up[:, :, :HID].reshape(layers, NET, 128, NHT, 128)
              .transpose(0, 3, 2, 1, 4).reshape(layers, NHT, 128, E))
    up_p = (w_up[:, :, HID:].reshape(layers, NET, 128, NHT, 128)
            .transpose(0, 3, 2, 1, 4).reshape(layers, NHT, 128, E))
    wup_p = _f16(np.concatenate([gate_p, up_p], axis=1))
    wdn_p = _f16(w_down.reshape(layers, NHT, 128, NET, 128)
                 .transpose(0, 3, 2, 1, 4).reshape(layers, NET, 128, HID))
    # vocab: [NVG, 128, NET*1024]; wvg[g, p, et*1024+c] = w_vocab[et*128+p, g*1024+c]
    # for g < 31; group 31 packs the 256-col tail at stride 256 per e-tile
    wv_pad = np.zeros((E, VP), np.float32)
    wv_pad[:, :V] = w_vocab
    wvg_p = np.asarray(wv_pad.reshape(NET, 128, NVG, 1024)
                       .transpose(2, 1, 0, 3).reshape(NVG, 128, NET * 1024),
                       dtype=np.float16)
    wvg_p[NVG - 1] = 0.0
    w31 = w_vocab[:, (NVG - 1) * 1024:V]            # [E, 256]
    wvg_p[NVG - 1, :, :NET * 256] = _f16(
        w31.reshape(NET, 128, 256).transpose(1, 0, 2).reshape(128, NET * 256))
    wvg_p = np.ascontiguousarray(wvg_p)
    ones = _f16(np.ones((128, 1), np.float32))
    onesr = _f16(np.ones((1, 128), np.float32))

    # ---- layer-0 kv blocks (host-precomputed, device-matching numerics) ----
    r0 = 1.0 / np.sqrt((x_full.astype(np.float64) ** 2).mean(axis=1) + EPS)
    n0 = (x_full * r0[:, None]).astype(np.float16).astype(np.float32)
    wk0 = wk[0].astype(np.float16).astype(np.float32)
    wv0 = wv[0].astype(np.float16).astype(np.float32)
    half = DK // 2
    kvblk = np.zeros((NCORES, 128, KV * 512), np.float16)
    for b in range(NCORES):
        nb_ = n0[b * SL:(b + 1) * SL]                       # [SL, E]
        cosb = cosT[:, b * SL:(b + 1) * SL]
        sinb = sinT[:, b * SL:(b + 1) * SL]
        for kvh in range(KV):
            kf = (nb_ @ wk0[:, kvh * DK:(kvh + 1) * DK]).T  # [DK, SL]
            t1 = np.empty_like(kf)
            t1[:half] = kf[half:] * sinb[:half]
            t1[half:] = kf[:half] * sinb[half:]
            kr = cosb * kf + t1
            vf = nb_ @ wv0[:, kvh * DK:(kvh + 1) * DK]      # [SL, DK]
            vp_ = vf.reshape(2, 128, 128).transpose(1, 0, 2).reshape(128, 256)
            kvblk[b, :, kvh * 512:kvh * 512 + 256] = kr.astype(np.float16)
            kvblk[b, :, kvh * 512 + 256:kvh * 512 + 512] = vp_.astype(np.float16)

    in_maps = []
    for c in range(NCORES):
        nb0 = max(0, c - 4)
        nbarr = np.zeros((1, 8), np.int32)
        nbarr[0, 0] = nb0
        in_maps.append({
            "x0": np.ascontiguousarray(x_full[SL * c:SL * (c + 1)].T),
            "cosT": np.ascontiguousarray(cosT[:, SL * c:SL * (c + 1)]),
            "sinT": np.ascontiguousarray(sinT[:, SL * c:SL * (c + 1)]),
            "masks": _f16(_make_masks(c)),
            "nbidx": nbarr,
            "kvg0": np.stack([kvblk[nb0 + i] for i in range(NB - 1)]),
            "kvself0": kvblk[c],
            "ones": ones,
            "onesr": onesr,
            "wq": wq_p,
            "wk": wk_p,
            "wv": wv_p,
            "wo": wo_p,
            "wup": wup_p,
            "wdn": wdn_p,
            "wvg": wvg_p,
        })
    return in_maps


def _run(inputs, trace=False, layers=L):
    global _BUILT
    if _BUILT is None or _BUILT[1] != layers:
        _BUILT = (build_graph(layers), layers)
    nc = _BUILT[0]
    in_maps = _prepare_inmaps(layers=layers, **inputs)
    res = run_bass_kernel_spmd(nc, in_maps, core_ids=list(range(NCORES)), trace=trace)
    logits = np.concatenate(
        [res.results[c]["out"].reshape(2 * 128, VP)[:, :V] for c in range(NCORES)],
        axis=0)
    return logits[None].astype(np.float32), res


def kernel(**inputs):
    logits, _ = _run(inputs, trace=False)
    return logits
